# revision 37
# baseline (speedup 1.0000x reference)
"""XNOR-Net BasicBlock (BN-sign-conv x2 + residual, training-mode BN) on 8 TRN2 cores.

Strategy (data-parallel on batch, 4 images/core):
  x is DMA-loaded ONCE (f32->fp16 cast in flight) and stays SBUF-resident
  for all three uses (BN1 stats, conv1 sign-prep, residual add).
  phase0: per-channel sum/sumsq of x (DVE reduce + ACT square) -> AllGather.
  conv k: s = (prev >= thr) as {0,1} fp8 on DVE (2x mode), pads 0.5 so the
          y = 2*y01 - rowsum(w) border correction is exact; 3x3 conv as 9
          DoubleRow fp8 matmuls per 8-row band (PE-saturated); epilogue
          t = 2a*y01 - a*rs (ACT, one PSUM read), u = max(t, t/a) (DVE,
          SBUF-only, accum -> sum u), sum(u^2) per quarter image split
          ACT/DVE. PE pstate kept hot through each collective by gated
          dummy-matmul chains (cold ramp costs ~6us/conv otherwise).
  phase3: th = K*u2+D (ACT), w = th+x (DVE tt 2x), out = max(w, a3*w)
          (ts 4x + tt max, ACT helping), fp16 output (host casts to f32).

Weights are binarized/laid out on host (tiny); all heavy tensors flow on device.
"""

import sys

sys.path.insert(0, "/opt/trn_rl_repo")

import numpy as np

import concourse.bacc as bacc
import concourse.mybir as mybir
import concourse.tile as tile
from concourse.bass_utils import run_bass_kernel_spmd

F32 = mybir.dt.float32
F16 = mybir.dt.float16
F8 = mybir.dt.float8e4
F8NP = mybir.dt.np(F8)
F16NP = mybir.dt.np(F16)

AF = mybir.ActivationFunctionType
OP = mybir.AluOpType
DR = mybir.MatmulPerfMode.DoubleRow

NCORES = 8
B, C, H, W = 32, 256, 56, 56
BL = B // NCORES          # images per core
HW = H * W                # 3136
HHW = HW // 2             # 1568 half-image chunk
PW = W + 2                # 58 padded width
PLANE = PW * PW           # 3364 padded plane (58 rows x 58 cols)
PLANE_PAD = 3392          # plane stride, %16 == 0
BAND = 8                  # output rows per matmul tile
NBAND = H // BAND         # 7
NFREE = BAND * PW         # 464 psum free size
NTOT = B * HW             # BN count (N*H*W over full batch)
EPS = 1e-5
OFFS = [(dh, dw) for dh in range(3) for dw in range(3)]

# cvec column indices
CV_B1, CV_A1, CV_SF1N, CV_SF1SQN, CV_B2, CV_ISF1, CV_A2, CV_SF2N, \
    CV_SF2SQN, CV_G3SF2, CV_G3, CV_B3V, CV_A3, CV_EPS, CV_IA1, CV_IA2, \
    CV_2A1, CV_2A2, CV_ARS1, CV_ARS2 = range(20)
CV_NCOLS = 20

_CACHE = {}


def _build():
    nc = bacc.Bacc(num_devices=NCORES)
    x_d = nc.declare_dram_parameter("x", [BL, C, H, W], F32, isOutput=False)
    w1_d = nc.declare_dram_parameter("w1s", [128, 2, 18 * 128], F8, isOutput=False)
    w2_d = nc.declare_dram_parameter("w2s", [128, 2, 18 * 128], F8, isOutput=False)
    cv_d = nc.declare_dram_parameter("cvec", [128, 2, CV_NCOLS], F32, isOutput=False)
    out_d = nc.declare_dram_parameter("out", [BL, C, H, W], F16, isOutput=True)

    # DRAM-side views: channel c -> (g = c // 128, p = c % 128)
    def x_view(n):
        return x_d[n].rearrange("(g p) h w -> p g (h w)", p=128)

    def out_view(n, g):
        return out_d[n].rearrange("(g p) h w -> p g h w", p=128)[:, g]

    with tile.TileContext(nc, num_cores=NCORES, pool_alloc_mode="queue") as tc:
        import contextlib

        es_u1 = contextlib.ExitStack()
        es_u2 = contextlib.ExitStack()
        es_ph0 = contextlib.ExitStack()
        with tc.tile_pool(name="consts", bufs=1) as cpool, \
                tc.tile_pool(name="weights", bufs=1) as wpool, \
                tc.tile_pool(name="spool", bufs=1) as spool, \
                tc.tile_pool(name="scr", bufs=3) as scrpool, \
                tc.tile_pool(name="psum", bufs=8, space="PSUM") as psum_pool, \
                tc.tile_pool(name="dram", bufs=1, space="DRAM") as dram_pool, \
                es_u2:

            # ---- persistent small tiles ----
            cvec = cpool.tile([128, 2, CV_NCOLS], F32, tag="cvec")
            st1 = cpool.tile([128, 32], F32, tag="st1")
            # per-conv stats: sum(u) per band tile (56 cols) + sum(u^2)
            # per quarter-image (16 cols)
            st2 = cpool.tile([128, 72], F32, tag="st2")
            st3 = cpool.tile([128, 72], F32, tag="st3")
            g1 = cpool.tile([128, 4], F32, tag="g1")
            g2 = cpool.tile([128, 4], F32, tag="g2")
            g3t = cpool.tile([128, 4], F32, tag="g3t")
            thr1 = cpool.tile([128, 2], F32, tag="thr1")
            thr2 = cpool.tile([128, 2], F32, tag="thr2")
            kvec = cpool.tile([128, 2], F32, tag="kvec")
            dvec = cpool.tile([128, 2], F32, tag="dvec")
            tmp_a = cpool.tile([128, 2], F32, tag="tmp_a")
            tmp_b = cpool.tile([128, 2], F32, tag="tmp_b")
            tmp_c = cpool.tile([128, 2], F32, tag="tmp_c")

            w1t = wpool.tile([128, 2, 18 * 128], F8, tag="w1t")
            w2t = wpool.tile([128, 2, 18 * 128], F8, tag="w2t")

            # x resident in SBUF as fp16 for the whole kernel
            x16 = cpool.tile([128, 2, BL * HW], F16, tag="x16")

            # pool open order fixes the release order (queue allocator):
            # u2 outlives u1; ph0 scratch dies first
            u2_pool = es_u2.enter_context(tc.tile_pool(name="u2", bufs=BL))
            u1_pool = es_u1.enter_context(tc.tile_pool(name="u1", bufs=BL))
            ph0_pool = es_ph0.enter_context(tc.tile_pool(name="ph0", bufs=2))

            s_tiles = [
                spool.tile([128, 2, PLANE_PAD], F8, tag="sa", name="sa"),
                spool.tile([128, 2, PLANE_PAD], F8, tag="sb", name="sb"),
            ]
            # PE warmup: dummy matmul input + dedicated psum bank. Chains of
            # dummy matmuls run during each collective so the PE pstate is
            # at max speed when the real conv starts (cold-start ramp costs
            # ~6us/conv otherwise).
            sdum = spool.tile([128, 2, NFREE], F8, tag="sdum", name="sdum")
            pdum = psum_pool.tile([128, NFREE], F32, tag="pdum", bufs=1,
                                  name="pdum")

            def warmup(gate, count):
                # gate: dummies read sdum, whose corner is rewritten by a
                # tiny op depending on `gate` -- so the chain starts only
                # once the gate value is ready, not at t=0.
                nc.vector.tensor_scalar(
                    sdum[0:1, 0, 0:2], gate[0:1, 0:2], scalar1=0.0,
                    scalar2=None, op0=OP.mult)
                for _ in range(count):
                    nc.tensor.matmul(
                        pdum[:], w1t[:, :, 0:128], sdum[:],
                        start=True, stop=True, perf_mode=DR)

            nc.sync.dma_start(cvec[:], cv_d[:])
            nc.sync.dma_start(w1t[:], w1_d[:])
            nc.sync.dma_start(w2t[:], w2_d[:])
            def s_plane(s, g):
                return s[:, g, 0:PLANE].rearrange("p (r w) -> p r w", w=PW)

            # =============== phase 0: x load (fp16 cast) + stats ===============
            # st1 col j = (k*2 + g)*4 + n   (k: 0=sum, 1=sumsq)
            for n in range(BL):
                for g in range(2):
                    for h in range(2):
                        c0 = h * HHW
                        chunk = x16[:, g, n * HW + c0:n * HW + c0 + HHW]
                        nc.gpsimd.dma_start(chunk, x_view(n)[:, g, c0:c0 + HHW])
                        j = (0 * 2 + g) * 8 + n * 2 + h
                        nc.vector.reduce_sum(
                            st1[:, j:j + 1], chunk, axis=mybir.AxisListType.X,
                        )
                        sqs = ph0_pool.tile([128, HHW], F16, tag="sqs",
                                            name=f"sqs_{n}_{g}_{h}")
                        j2 = (1 * 2 + g) * 8 + n * 2 + h
                        nc.scalar.activation(
                            sqs[:], chunk, AF.Square, bias=0.0, scale=1.0,
                            accum_out=st1[:, j2:j2 + 1],
                        )

            # s tiles hold {0,1} signs; pad cells stay 0.5 so the
            # y = 2*y01 - rowsum(w) correction is exact at borders
            # (0.5 -> "0" contribution). Emitted after the x loads so the
            # Pool queue issues the SWDGE casting loads first.
            for s in s_tiles:
                nc.gpsimd.memset(s[:], 0.5)
            nc.gpsimd.memset(sdum[:], 1.0)

            # reduce st1 [128, (kg) 4] -> r1 [128, 4], AllGather -> g1
            r1 = cpool.tile([128, 4], F32, tag="r1")
            nc.vector.reduce_sum(
                r1[:].rearrange("p (a b) -> p a b", b=1),
                st1[:].rearrange("p (kg t) -> p kg t", t=8),
                axis=mybir.AxisListType.X,
            )
            ar1_i = dram_pool.tile([128, 4], F32, tag="ar1_i")
            ar1_o = dram_pool.tile([NCORES, 128, 4], F32, tag="ar1_o", addr_space="Shared")
            nc.sync.dma_start(ar1_i[:], r1[:])
            warmup(r1, 195)
            nc.gpsimd.collective_compute(
                "AllGather", OP.bypass, replica_groups=[list(range(NCORES))],
                ins=[ar1_i[:].opt()], outs=[ar1_o[:].opt()],
            )
            gth1 = cpool.tile([128, 4, NCORES], F32, tag="gth1")
            nc.sync.dma_start(gth1[:], ar1_o[:].rearrange("r p k -> p k r"))
            nc.vector.reduce_sum(
                g1[:].rearrange("p (a b) -> p a b", b=1), gth1[:],
                axis=mybir.AxisListType.X,
            )
            es_ph0.close()

            # ---- BN1 threshold: thr1 = m1 - B1*std1 ----
            g1v = g1[:].rearrange("p (k g) -> p k g", k=2)
            nc.vector.tensor_scalar_mul(tmp_a[:], g1v[:, 0], 1.0 / NTOT)      # m1
            nc.vector.tensor_scalar_mul(tmp_b[:], g1v[:, 1], 1.0 / NTOT)      # E[x^2]
            nc.vector.scalar_tensor_tensor(                                   # v1 = E - m^2
                tmp_c[:], tmp_a[:], -1.0, tmp_a[:], op0=OP.mult, op1=OP.mult,
            )
            nc.vector.tensor_add(tmp_c[:], tmp_c[:], tmp_b[:])                # v1
            nc.scalar.activation(tmp_b[:], tmp_c[:], AF.Sqrt, bias=cvec[:, 0, CV_EPS:CV_EPS + 1], scale=1.0)  # std1
            nc.vector.tensor_mul(tmp_c[:], tmp_b[:], cvec[:, :, CV_B1])       # B1*std1
            nc.vector.tensor_sub(thr1[:], tmp_a[:], tmp_c[:])                 # m1 - B1*std1

            # =============== conv pass helper ===============
            def conv_pass(widx, wt, stats, prep, u_pool):
                """One binary conv over all images.

                prep(n, s) emits the {0,1} threshold-compare of image n into
                the s tile interior (pads stay 0.5). psum y01 is corrected to
                t = a*yhat = 2a*y01 - a*rowsum(w) inside the ACT t-op.
                stats: sum(u) col = cc*28 + n*7 + b; sum(u^2) col =
                56 + ((cc*4 + n)*2 + q) per quarter image.
                Returns list of u tiles [128, 2, H, W] fp16 (prelu, y-units).
                """
                a2_col = CV_2A1 if widx == 0 else CV_2A2
                ars_col = CV_ARS1 if widx == 0 else CV_ARS2
                ia_col = CV_IA1 if widx == 0 else CV_IA2
                u_tiles = []
                prep(0, s_tiles[0])
                for n in range(BL):
                    s = s_tiles[n % 2]
                    # next image's sign-prep chunks are interleaved between
                    # early bands so the DVE queue never bursts
                    nxt = (iter(prep(n + 1, s_tiles[(n + 1) % 2], emit=False))
                           if n + 1 < BL else iter(()))
                    ut = u_pool.tile([128, 2, H, W], F16, tag=f"u{widx}", name=f"u{widx}_{n}")
                    u_tiles.append(ut)
                    for cc in range(2):
                        for b in range(NBAND):
                            if cc == 0 and b % 2 == 0:
                                for fn in (next(nxt, None),):
                                    if fn is not None:
                                        fn()
                            pt = psum_pool.tile(
                                [128, NFREE], F32, tag="pt", bufs=7,
                                name=f"pt{widx}_{n}_{cc}_{b}",
                            )
                            for o, (dh, dw) in enumerate(OFFS):
                                start = (b * BAND + dh) * PW + dw
                                nc.tensor.matmul(
                                    pt[:],
                                    wt[:, :, (o * 2 + cc) * 128:(o * 2 + cc + 1) * 128],
                                    s[:, :, start:start + NFREE],
                                    start=(o == 0), stop=(o == 8),
                                    perf_mode=DR,
                                )
                            pv = pt[:].rearrange("p (r w) -> p r w", w=PW)[:, :, 0:W]
                            j0 = cc * 28 + n * NBAND + b
                            us = ut[:, cc, b * BAND:(b + 1) * BAND, :]
                            # t = 2a*y01 - a*rs = a*yhat (ACT, single PSUM read)
                            tt = scrpool.tile([128, BAND, W], F16, tag="tt", bufs=10)
                            nc.scalar.activation(
                                tt[:], pv, AF.Identity,
                                bias=cvec[:, cc, ars_col:ars_col + 1],
                                scale=cvec[:, cc, a2_col:a2_col + 1],
                            )
                            # u = prelu(yhat) = max(t, t/a) (DVE, SBUF only);
                            # accum -> sum(u)
                            nc.vector.scalar_tensor_tensor(
                                us, tt[:], cvec[:, cc, ia_col:ia_col + 1], tt[:],
                                op0=OP.mult, op1=OP.max,
                                accum_out=stats[:, j0:j0 + 1],
                            )
                            # sum(u^2) per quarter image, emitted as soon as
                            # the last band of the quarter is written
                            if b == 3 or b == NBAND - 1:
                                q = 0 if b == 3 else 1
                                uq = ut[:, cc, q * 28:(q + 1) * 28, :].rearrange(
                                    "p h w -> p (h w)")
                                j1 = 56 + (cc * BL + n) * 2 + q
                                sq = scrpool.tile([128, 28 * W], F32, tag="sq",
                                                  bufs=2)
                                if q == 0 if n == BL - 1 else not (cc == 1 and q == 1):
                                    nc.scalar.activation(
                                        sq[:], uq, AF.Square, bias=0.0,
                                        scale=1.0,
                                        accum_out=stats[:, j1:j1 + 1],
                                    )
                                else:
                                    nc.vector.scalar_tensor_tensor(
                                        sq[:], uq, 1.0, uq,
                                        op0=OP.mult, op1=OP.mult,
                                        accum_out=stats[:, j1:j1 + 1],
                                    )

                return u_tiles

            # =============== conv1 ===============
            def prep1(n, s, emit=True):
                xv = x16[:, :, n * HW:(n + 1) * HW]
                thunks = []
                for r0, r1_ in ((0, 28), (28, 56)):
                    for g in range(2):
                        # {0,1} compare on DVE (2x mode); pads remain 0.5
                        def fn(r0=r0, r1_=r1_, g=g):
                            nc.vector.tensor_scalar(
                                s_plane(s, g)[:, 1 + r0:1 + r1_, 1:57],
                                xv[:, g].rearrange("p (h w) -> p h w", w=W)[:, r0:r1_],
                                scalar1=thr1[:, g:g + 1], scalar2=None,
                                op0=OP.is_ge,
                            )
                        if emit:
                            fn()
                        else:
                            thunks.append(fn)
                return thunks

            u1 = conv_pass(0, w1t, st2, prep1, u1_pool)

            # reduce st2 -> r2 (sum block: 2x28 cols; sumsq block: 2x8 cols)
            r2 = cpool.tile([128, 4], F32, tag="r2")
            nc.vector.reduce_sum(
                r2[:, 0:2].rearrange("p (a b) -> p a b", b=1),
                st2[:, 0:56].rearrange("p (c t) -> p c t", t=28),
                axis=mybir.AxisListType.X,
            )
            nc.vector.reduce_sum(
                r2[:, 2:4].rearrange("p (a b) -> p a b", b=1),
                st2[:, 56:72].rearrange("p (c t) -> p c t", t=8),
                axis=mybir.AxisListType.X,
            )
            ar2_i = dram_pool.tile([128, 4], F32, tag="ar2_i")
            ar2_o = dram_pool.tile([NCORES, 128, 4], F32, tag="ar2_o", addr_space="Shared")
            nc.sync.dma_start(ar2_i[:], r2[:])
            warmup(u1[BL - 1][:, 1, H - 1], 217)
            nc.gpsimd.collective_compute(
                "AllGather", OP.bypass, replica_groups=[list(range(NCORES))],
                ins=[ar2_i[:].opt()], outs=[ar2_o[:].opt()],
            )
            gth2 = cpool.tile([128, 4, NCORES], F32, tag="gth2")
            nc.sync.dma_start(gth2[:], ar2_o[:].rearrange("r p k -> p k r"))
            nc.vector.reduce_sum(
                g2[:].rearrange("p (a b) -> p a b", b=1), gth2[:],
                axis=mybir.AxisListType.X,
            )

            # ---- BN2 threshold in u1 units ----
            g2v = g2[:].rearrange("p (k c) -> p k c", k=2)
            nc.vector.tensor_mul(tmp_a[:], g2v[:, 0], cvec[:, :, CV_SF1N])    # m2
            nc.vector.tensor_mul(tmp_b[:], g2v[:, 1], cvec[:, :, CV_SF1SQN])  # E[p1^2]
            nc.vector.scalar_tensor_tensor(
                tmp_c[:], tmp_a[:], -1.0, tmp_a[:], op0=OP.mult, op1=OP.mult,
            )
            nc.vector.tensor_add(tmp_c[:], tmp_c[:], tmp_b[:])                # v2
            nc.scalar.activation(tmp_b[:], tmp_c[:], AF.Sqrt, bias=cvec[:, 0, CV_EPS:CV_EPS + 1], scale=1.0)  # std2
            nc.vector.tensor_mul(tmp_c[:], tmp_b[:], cvec[:, :, CV_B2])       # B2*std2
            nc.vector.tensor_sub(tmp_a[:], tmp_a[:], tmp_c[:])                # t2 = m2 - B2*std2
            nc.vector.tensor_mul(thr2[:], tmp_a[:], cvec[:, :, CV_ISF1])      # theta (u units)

            # =============== conv2 ===============
            def prep2(n, s, emit=True):
                thunks = []
                for r0, r1_ in ((0, 28), (28, 56)):
                    for g in range(2):
                        def fn(r0=r0, r1_=r1_, g=g):
                            nc.vector.tensor_scalar(
                                s_plane(s, g)[:, 1 + r0:1 + r1_, 1:57],
                                u1[n][:, g, r0:r1_, :],
                                scalar1=thr2[:, g:g + 1], scalar2=None,
                                op0=OP.is_ge,
                            )
                        if emit:
                            fn()
                        else:
                            thunks.append(fn)
                return thunks

            u2 = conv_pass(1, w2t, st3, prep2, u2_pool)

            # u1 fully consumed by prep2; release its pool so the queue
            # allocator can reuse the region for phase-3 tiles
            es_u1.close()

            # reduce st3 -> r3, AllReduce -> g3t
            r3 = cpool.tile([128, 4], F32, tag="r3")
            nc.vector.reduce_sum(
                r3[:, 0:2].rearrange("p (a b) -> p a b", b=1),
                st3[:, 0:56].rearrange("p (c t) -> p c t", t=28),
                axis=mybir.AxisListType.X,
            )
            nc.vector.reduce_sum(
                r3[:, 2:4].rearrange("p (a b) -> p a b", b=1),
                st3[:, 56:72].rearrange("p (c t) -> p c t", t=8),
                axis=mybir.AxisListType.X,
            )
            ar3_i = dram_pool.tile([128, 4], F32, tag="ar3_i")
            ar3_o = dram_pool.tile([NCORES, 128, 4], F32, tag="ar3_o", addr_space="Shared")
            nc.sync.dma_start(ar3_i[:], r3[:])
            nc.gpsimd.collective_compute(
                "AllGather", OP.bypass, replica_groups=[list(range(NCORES))],
                ins=[ar3_i[:].opt()], outs=[ar3_o[:].opt()],
            )
            gth3 = cpool.tile([128, 4, NCORES], F32, tag="gth3")
            nc.sync.dma_start(gth3[:], ar3_o[:].rearrange("r p k -> p k r"))
            nc.vector.reduce_sum(
                g3t[:].rearrange("p (a b) -> p a b", b=1), gth3[:],
                axis=mybir.AxisListType.X,
            )

            # ---- BN3 affine: K = g3*sf2*rstd3, D = b3 - m3*g3*rstd3 ----
            g3v = g3t[:].rearrange("p (k c) -> p k c", k=2)
            nc.vector.tensor_mul(tmp_a[:], g3v[:, 0], cvec[:, :, CV_SF2N])    # m3
            nc.vector.tensor_mul(tmp_b[:], g3v[:, 1], cvec[:, :, CV_SF2SQN])  # E[p2^2]
            nc.vector.scalar_tensor_tensor(
                tmp_c[:], tmp_a[:], -1.0, tmp_a[:], op0=OP.mult, op1=OP.mult,
            )
            nc.vector.tensor_add(tmp_c[:], tmp_c[:], tmp_b[:])                # v3
            nc.scalar.activation(tmp_b[:], tmp_c[:], AF.Sqrt, bias=cvec[:, 0, CV_EPS:CV_EPS + 1], scale=1.0)  # std3
            nc.vector.reciprocal(tmp_c[:], tmp_b[:])                          # rstd3
            nc.vector.tensor_mul(kvec[:], tmp_c[:], cvec[:, :, CV_G3SF2])     # K
            nc.vector.tensor_mul(tmp_a[:], tmp_a[:], cvec[:, :, CV_G3])       # m3*g3
            nc.vector.tensor_mul(tmp_a[:], tmp_a[:], tmp_c[:])                # m3*g3*rstd3
            nc.vector.tensor_sub(dvec[:], cvec[:, :, CV_B3V], tmp_a[:])       # D

            # =============== phase 3: out = prelu(K*u2 + D + x, a3) ===============
            with tc.tile_pool(name="ph3", bufs=2) as p3pool:
                unit = 0
                for n in range(BL):
                    for g in range(2):
                        th = p3pool.tile([128, HW], F16, tag="th", name=f"th_{n}_{g}")
                        aw = p3pool.tile([128, HW], F16, tag="aw", name=f"aw_{n}_{g}")
                        ot = p3pool.tile([128, HW], F16, tag="ot", name=f"ot_{n}_{g}")
                        # th = K*u2 + D (ACT)
                        nc.scalar.activation(
                            th[:], u2[n][:, g].rearrange("p h w -> p (h w)"),
                            AF.Identity,
                            bias=dvec[:, g:g + 1], scale=kvec[:, g:g + 1],
                        )
                        # w = th + x (DVE tt, fp16 2x mode)
                        wv = p3pool.tile([128, HW], F16, tag="wv",
                                         name=f"wv_{n}_{g}")
                        nc.vector.tensor_add(
                            wv[:], th[:], x16[:, g, n * HW:(n + 1) * HW])
                        # out = max(w, a3*w): aw on ACT for 3 of 8 units
                        # (ACT has slack), else DVE ts 4x; max on DVE
                        if unit % 3 == 1:
                            nc.scalar.activation(
                                aw[:], wv[:], AF.Identity, bias=0.0,
                                scale=cvec[:, g, CV_A3:CV_A3 + 1])
                        else:
                            nc.vector.tensor_scalar(
                                aw[:], wv[:], scalar1=cvec[:, g, CV_A3:CV_A3 + 1],
                                scalar2=None, op0=OP.mult)
                        if unit == 7:
                            for hh in range(2):
                                sl = slice(hh * HHW, (hh + 1) * HHW)
                                nc.vector.tensor_tensor(
                                    ot[:, sl], aw[:, sl], wv[:, sl], op=OP.max)
                                nc.sync.dma_start(
                                    out_view(n, g)[:, hh * 28:(hh + 1) * 28, :],
                                    ot[:, sl].rearrange("p (h w) -> p h w", w=W))
                        else:
                            nc.vector.tensor_tensor(ot[:], aw[:], wv[:], op=OP.max)
                            nc.sync.dma_start(
                                out_view(n, g),
                                ot[:].rearrange("p (h w) -> p h w", w=W))
                        unit += 1

    nc.compile()
    return nc


def _host_prep(inputs):
    x = np.ascontiguousarray(np.asarray(inputs["x"], dtype=np.float32))
    w1 = np.asarray(inputs["w1"], dtype=np.float32)
    w2 = np.asarray(inputs["w2"], dtype=np.float32)

    def wprep(w):
        ws = np.sign(w).astype(np.float32)  # [co, ci, kh, kw]
        sf = np.abs(w).mean(axis=(1, 2, 3)).astype(np.float32)  # [256]
        rs = ws.sum(axis=(1, 2, 3)).astype(np.float32)          # rowsum per co
        arr = np.empty((128, 2, 18, 128), dtype=np.float32)
        for o, (dh, dw) in enumerate(OFFS):
            for cc in range(2):
                t = ws[cc * 128:(cc + 1) * 128, :, dh, dw]  # [m, ci]
                # arr[p, g, blk, m] = t[m, g*128 + p]
                arr[:, :, o * 2 + cc, :] = t.T.reshape(2, 128, 128).transpose(1, 0, 2)
        return arr.reshape(128, 2, 18 * 128).astype(F8NP), sf, rs

    w1s, sf1, rs1 = wprep(w1)
    w2s, sf2, rs2 = wprep(w2)

    def vec(v):
        return np.asarray(v, dtype=np.float32).reshape(2, 128).T  # [p, g]

    g1v, b1v = inputs["g1"], inputs["b1"]
    g2v, b2v = inputs["g2"], inputs["b2"]
    g3v, b3v = inputs["g3"], inputs["b3"]
    a1, a2, a3 = inputs["a1"], inputs["a2"], inputs["a3"]

    cvec = np.zeros((128, 2, CV_NCOLS), dtype=np.float32)
    cvec[:, :, CV_B1] = vec(np.asarray(b1v) / np.asarray(g1v))
    cvec[:, :, CV_A1] = vec(np.asarray(a1))
    cvec[:, :, CV_SF1N] = vec(sf1 / NTOT)
    cvec[:, :, CV_SF1SQN] = vec(sf1 * sf1 / NTOT)
    cvec[:, :, CV_B2] = vec(np.asarray(b2v) / np.asarray(g2v))
    cvec[:, :, CV_ISF1] = vec(1.0 / sf1)
    cvec[:, :, CV_A2] = vec(np.asarray(a2))
    cvec[:, :, CV_SF2N] = vec(sf2 / NTOT)
    cvec[:, :, CV_SF2SQN] = vec(sf2 * sf2 / NTOT)
    cvec[:, :, CV_G3SF2] = vec(np.asarray(g3v) * sf2)
    cvec[:, :, CV_G3] = vec(np.asarray(g3v))
    cvec[:, :, CV_B3V] = vec(np.asarray(b3v))
    cvec[:, :, CV_A3] = vec(np.asarray(a3))
    cvec[:, :, CV_EPS] = EPS
    cvec[:, :, CV_IA1] = vec(1.0 / np.asarray(a1))
    cvec[:, :, CV_IA2] = vec(1.0 / np.asarray(a2))
    cvec[:, :, CV_2A1] = vec(2.0 * np.asarray(a1))
    cvec[:, :, CV_2A2] = vec(2.0 * np.asarray(a2))
    cvec[:, :, CV_ARS1] = vec(-np.asarray(a1) * rs1)
    cvec[:, :, CV_ARS2] = vec(-np.asarray(a2) * rs2)

    return x, w1s, w2s, cvec


def run(inputs, trace=False):
    x, w1s, w2s, cvec = _host_prep(inputs)
    if "nc" not in _CACHE:
        _CACHE["nc"] = _build()
    nc = _CACHE["nc"]
    in_maps = [
        {"x": x[i * BL:(i + 1) * BL], "w1s": w1s, "w2s": w2s, "cvec": cvec}
        for i in range(NCORES)
    ]
    res = run_bass_kernel_spmd(nc, in_maps, list(range(NCORES)), trace=trace)
    out = np.concatenate([res.results[i]["out"] for i in range(NCORES)], axis=0)
    return out.astype(np.float32), res


def kernel(**inputs):
    out, _ = run(inputs, trace=False)
    return out


if __name__ == "__main__":
    # build-only check
    _build()
    print("BUILD OK")
